# revision 13
# baseline (speedup 1.0000x reference)
"""Ragged-segment attention for Trainium2 (8 NeuronCores, SPMD), bin-dense.

Per-segment masking/softmax structure is folded into a host-built low-rank
additive mask applied with ONE matmul per bin:
    mask[q,k] = (kb[k] + NEG) * 1  +  sum_s (-NEG) * 1_s[q] 1_s[k]
so scores/softmax/out are all dense [128 x 128] bin ops and segments pack at
arbitrary offsets (first-fit decreasing, ~99% dense bins).

v3 (pure-PE stream, no transposes):
  - C^T is packed on the host (cpkt, fp16) and DMA'd directly.
  - scores are computed TRANSPOSED per bin: sc_T[k,q] = sum_e u^T[e,k] C^T[e,q]
    (same tiles as the u-matmul, swapped matmul roles), so the exp output is
    already in out-matmul orientation -- no PE/DVE transposes at all.
  - softmax runs WITHOUT max-subtraction: scores are bounded (|s| < ~80,
    verified on host), so exp fits fp32/bf16 range; exp output is bf16 and
    the out-matmul runs bf16 x bf16 (C also packed bf16 for that matmul).
    Row sums come from a 1-column ones matmul reusing the same stationary.
  - n_bins is not padded to a multiple of GROUP (remainder group).
  - loads are prefetched 2 iterations ahead so u-matmuls never wait on DMA.
PE stream per 4-bin group: 16 u-matmuls (512 rows) + 4x5 score matmuls
(128 rows) + 4x(512+1) out matmuls = ~12800 cycles ~ 5.34us @ 2.4GHz.
"""
import numpy as np
import ml_dtypes

import concourse.bacc as bacc
import concourse.mybir as mybir
import concourse.tile as tile
from concourse.bass_utils import run_bass_kernel_spmd

F32 = mybir.dt.float32
FP16 = mybir.dt.float16
BF16 = mybir.dt.bfloat16

N_CORES = 8
D = 512
BIN = 128
GROUP = 4

LAST_RESULTS = {}


def _plan(lengths, mode="fp16"):
    S = len(lengths)
    n_slots = S // N_CORES
    order = np.argsort(-lengths, kind="stable")
    seg_ids = [[int(order[N_CORES * j + c]) for j in range(n_slots)]
               for c in range(N_CORES)]
    slot_len = [int(lengths[order[N_CORES * j]]) for j in range(n_slots)]

    bins = []   # (used-token count, n_segs) per bin
    slots = []  # (bin, off, L)
    for j, L in enumerate(slot_len):
        bi = next((i for i, (used, ns) in enumerate(bins)
                   if used + L <= BIN and ns < 31), None)
        if bi is None:
            bins.append((0, 0))
            bi = len(bins) - 1
        used, ns = bins[bi]
        slots.append((bi, used, L))
        bins[bi] = (used + L, ns + 1)
    return slots, len(bins), seg_ids


def _groups(n_bins):
    gs = []
    b = 0
    while b < n_bins:
        gs.append((b, min(GROUP, n_bins - b)))
        b += GROUP
    return gs


def _mask_layout(slots, n_bins):
    by_bin = [[] for _ in range(n_bins)]
    for bi, off, L in slots:
        by_bin[bi].append((off, L))
    kmask = [len(by_bin[b]) + 1 for b in range(n_bins)]
    assert max(kmask) <= 32
    return by_bin, kmask


def _build(slots, n_bins, mode="fp16", repeat=1):
    assert mode == "fp16"
    nc = bacc.Bacc("TRN2", target_bir_lowering=False)
    T = n_bins * BIN
    groups = _groups(n_bins)
    n_groups = len(groups)

    by_bin, kmask = _mask_layout(slots, n_bins)

    # cpk is the out-matmul operand (bf16 to match the bf16 exp weights);
    # cpkt is C^T for the u/score matmuls (fp16: score precision matters).
    cpk = nc.dram_tensor("cpk", [T, D], BF16, kind="ExternalInput")
    cpkt = nc.dram_tensor("cpkt", [D, T], FP16, kind="ExternalInput")
    wt = nc.dram_tensor("wt", [128, 4 * D], FP16, kind="ExternalInput")
    bvec = nc.dram_tensor("bvec", [128, 4], F32, kind="ExternalInput")
    # per-group mask rows: bin i of a group at partitions [32i, 32i+km)
    msk = nc.dram_tensor("msk", [n_groups * 128, 2 * 128], FP16,
                         kind="ExternalInput")
    opk = nc.dram_tensor("opk", [T, D], FP16, kind="ExternalOutput")

    ones = nc.inline_tensor(np.ones((128, 1), dtype=ml_dtypes.bfloat16),
                            name="ones")

    with tile.TileContext(nc) as tc:
        with (
            tc.tile_pool(name="const", bufs=1) as cpool,
            tc.tile_pool(name="cb", bufs=3) as cbp,
            tc.tile_pool(name="grp", bufs=3) as grp,
            tc.tile_pool(name="seg", bufs=6) as segp,
            tc.tile_pool(name="stat", bufs=8) as statp,
            tc.tile_pool(name="outp", bufs=3) as outp,
            tc.tile_pool(name="ups", bufs=3, space="PSUM") as ups,
            tc.tile_pool(name="scps", bufs=2, space="PSUM") as scps,
            tc.tile_pool(name="ops", bufs=3, space="PSUM") as opsp,
        ):
            wt_sb = cpool.tile([128, 4, D], FP16, tag="wt")
            b_sb = cpool.tile([128, 4], F32, tag="b")
            ones_sb = cpool.tile([128, 1], BF16, tag="ones")
            msk_sb = cpool.tile([128, n_groups, 2, 128], FP16, tag="msk")
            nc.sync.dma_start(wt_sb[:], wt.ap().rearrange("p (c e) -> p c e", c=4))
            nc.sync.dma_start(b_sb[:], bvec[:])
            nc.sync.dma_start(ones_sb[:], ones[:])
            nc.sync.dma_start(
                msk_sb[:],
                msk.ap().rearrange("(g r) (t p) -> r g t p", g=n_groups, t=2))

            cpk_v = cpk.ap().rearrange("(b p) d -> p b d", p=BIN)
            cpkt_v = cpkt.ap().rearrange("(c p) t -> p c t", p=128)
            opk_v = opk.ap().rearrange("(b p) d -> p b d", p=BIN)

            def load_group(gi):
                b0, gs = groups[gi]
                cg = cbp.tile([128, gs, D], BF16, tag="cg")
                nc.sync.dma_start(cg[:], cpk_v[:, b0:b0 + gs, :])
                ct = grp.tile([128, 4, gs * BIN], FP16, tag="ct")
                nc.sync.dma_start(
                    ct[:], cpkt_v[:, :, b0 * BIN:(b0 + gs) * BIN])
                return {"gi": gi, "cg": cg, "ct": ct}

            def u_group(st):
                _, gs = groups[st["gi"]]
                ct = st["ct"]
                ut = grp.tile([128, 4, gs * BIN], FP16, tag="ut")
                for c in range(4):
                    ups_t = ups.tile([128, gs * BIN], F32, tag="ups")
                    for k in range(4):
                        nc.tensor.matmul(
                            ups_t[:], wt_sb[:, k, c * 128:(c + 1) * 128],
                            ct[:, k, :], start=(k == 0), stop=(k == 3))
                    nc.scalar.activation(
                        ut[:, c, :], ups_t[:],
                        mybir.ActivationFunctionType.Tanh, bias=b_sb[:, c:c + 1])
                st["ut"] = ut

            def bin_scores(st, i):
                gi = st["gi"]
                b0, gs = groups[gi]
                b = b0 + i
                if not by_bin[b]:
                    return
                ct, ut = st["ct"], st["ut"]
                km = kmask[b]
                # transposed scores: sc_T[k, q] = sum_e u^T[e,k] C^T[e,q]
                sc = scps.tile([128, 128], F32, tag="sc")
                for c in range(4):
                    nc.tensor.matmul(
                        sc[:], ut[:, c, i * BIN:(i + 1) * BIN],
                        ct[:, c, i * BIN:(i + 1) * BIN],
                        start=(c == 0), stop=False)
                nc.tensor.matmul(sc[:], msk_sb[32 * i:32 * i + km, gi, 1, :],
                                 msk_sb[32 * i:32 * i + km, gi, 0, :],
                                 start=False, stop=True,
                                 tile_position=(32 * i, 0))
                # max-free softmax numerator, already [k, q] for the out-matmul
                attn = segp.tile([128, 128], BF16, tag="attn")
                nc.scalar.activation(
                    attn[:], sc[:], mybir.ActivationFunctionType.Exp)
                st[("attn", i)] = attn

            def bin_out(st, i, og, use_act_copy):
                if ("attn", i) not in st:
                    return
                attn = st.pop(("attn", i))
                cg = st["cg"]
                sums = scps.tile([128, 1], F32, tag="sc")
                nc.tensor.matmul(sums[:], attn[:], ones_sb[:],
                                 start=True, stop=True)
                ops_t = opsp.tile([128, D], F32, tag="ops")
                nc.tensor.matmul(ops_t[:], attn[:], cg[:, i, :],
                                 start=True, stop=True)
                recip = statp.tile([128, 1], F32, tag="recip")
                nc.vector.reciprocal(recip[:], sums[:])
                # normalize rows by 1/sum during the psum->sbuf copy
                if use_act_copy:
                    nc.scalar.activation(og[:, i, :], ops_t[:],
                                         mybir.ActivationFunctionType.Copy,
                                         scale=recip[:])
                else:
                    nc.vector.tensor_scalar_mul(og[:, i, :], ops_t[:], recip[:])

            def finish_group(st):
                gi = st["gi"]
                b0, gs = groups[gi]
                og = outp.tile([128, gs, D], FP16, tag="og")
                for i in range(gs):
                    bin_out(st, i, og, use_act_copy=(i % 2 == 0))
                # ACT HWDGE queue: keeps the blocking store off the SP
                # load queue (HWDGE DMAs issue in order per engine queue)
                nc.scalar.dma_start(opk_v[:, b0:b0 + gs, :], og[:])

            # software pipeline over groups, prefetch depth 2:
            #   iter t: load group t | u-matmuls group t-1 | scores/out t-2
            niter = repeat * n_groups
            states = {}
            for it in range(niter + 2):
                st_s = states.pop(it - 2, None)
                if st_s is not None:
                    for i in range(groups[st_s["gi"]][1]):
                        bin_scores(st_s, i)
                st_u = states.get(it - 1)
                if st_u is not None:
                    u_group(st_u)
                if st_s is not None:
                    finish_group(st_s)
                if it < niter:
                    states[it] = load_group(it % n_groups)

    nc.compile()
    return nc


def _host_arrays(slots, n_bins, seg_ids, lengths, context, W, b, mode="fp16"):
    NEG = -30000.0
    T = n_bins * BIN
    by_bin2 = [[] for _ in range(n_bins)]
    for j, (bi, off, L) in enumerate(slots):
        by_bin2[bi].append((j, off, L))
    n_groups = len(_groups(n_bins))

    wt = np.ascontiguousarray(
        W.T.reshape(4, 128, D).transpose(1, 0, 2).reshape(128, 4 * D)
    ).astype(np.float16)
    bvec = np.ascontiguousarray(b.reshape(4, 128).T).astype(np.float32)

    in_maps = []
    for c in range(N_CORES):
        cpk = np.zeros((T, D), np.float16)
        kb = np.full(T, NEG, np.float32)
        for j, (bi, off, _L) in enumerate(slots):
            s = seg_ids[c][j]
            n = int(lengths[s])
            r0 = bi * BIN + off
            cpk[r0:r0 + n] = context[s, :n].astype(np.float16)
            kb[r0:r0 + n] = 0.0
        msk = np.zeros((n_groups * 128, 2, 128), np.float32)
        for bb in range(n_bins):
            r0 = (bb // GROUP) * 128 + 32 * (bb % GROUP)
            msk[r0, 0] = 1.0
            msk[r0, 1] = kb[bb * BIN:(bb + 1) * BIN] + NEG
            for r, (_j, off, L) in enumerate(by_bin2[bb]):
                msk[r0 + 1 + r, 0, off:off + L] = 1.0
                msk[r0 + 1 + r, 1, off:off + L] = -NEG
        in_maps.append({"cpk": cpk.astype(ml_dtypes.bfloat16),
                        "cpkt": np.ascontiguousarray(cpk.T),
                        "wt": wt, "bvec": bvec,
                        "msk": msk.reshape(n_groups * 128, 256).astype(np.float16)})
    return in_maps


_CACHE = {}


def kernel(context, lengths, W, b, mode="fp16"):
    context = np.asarray(context, dtype=np.float32)
    lengths = np.asarray(lengths, dtype=np.int32)
    W = np.asarray(W, dtype=np.float32)
    b = np.asarray(b, dtype=np.float32)
    S, Lmax, Din = context.shape

    slots, n_bins, seg_ids = _plan(lengths, mode)
    key = (tuple(slots), n_bins, mode)
    if key in _CACHE:
        nc = _CACHE[key]
    else:
        nc = _build(slots, n_bins, mode)
        _CACHE[key] = nc

    in_maps = _host_arrays(slots, n_bins, seg_ids, lengths, context, W, b, mode)
    res = run_bass_kernel_spmd(nc, in_maps, list(range(N_CORES)))
    LAST_RESULTS["exec_time_ns"] = res.exec_time_ns

    out = np.zeros((S, Lmax, D), np.float32)
    for c in range(N_CORES):
        opk = res.results[c]["opk"].astype(np.float32)
        for j, (bi, off, _L) in enumerate(slots):
            s = seg_ids[c][j]
            n = int(lengths[s])
            r0 = bi * BIN + off
            out[s, :n] = opk[r0:r0 + n]
    return out


# revision 15
# speedup vs baseline: 1.3599x; 1.3599x over previous
"""Ragged-segment attention for Trainium2 (8 NeuronCores, SPMD), bin-dense.

Per-segment masking/softmax structure is folded into a host-built low-rank
additive mask applied with ONE matmul per bin:
    mask[q,k] = (kb[k] + NEG) * 1  +  sum_s (-NEG) * 1_s[q] 1_s[k]
so scores/softmax/out are all dense [128 x 128] bin ops and segments pack at
arbitrary offsets (first-fit decreasing, ~99% dense bins).

v3 (pure-PE stream, no transposes):
  - C^T is packed on the host (cpkt, fp16) and DMA'd directly.
  - scores are computed TRANSPOSED per bin: sc_T[k,q] = sum_e u^T[e,k] C^T[e,q]
    (same tiles as the u-matmul, swapped matmul roles), so the exp output is
    already in out-matmul orientation -- no PE/DVE transposes at all.
  - softmax runs WITHOUT max-subtraction: scores are bounded (|s| < ~80,
    verified on host), so exp fits fp32/bf16 range; exp output is bf16 and
    the out-matmul runs bf16 x bf16 (C also packed bf16 for that matmul).
    Row sums come from a 1-column ones matmul reusing the same stationary.
  - n_bins is not padded to a multiple of GROUP (remainder group).
  - loads are prefetched 2 iterations ahead so u-matmuls never wait on DMA.
PE stream per 4-bin group: 16 u-matmuls (512 rows) + 4x5 score matmuls
(128 rows) + 4x(512+1) out matmuls = ~12800 cycles ~ 5.34us @ 2.4GHz.
"""
import numpy as np
import ml_dtypes

import concourse.bacc as bacc
import concourse.mybir as mybir
import concourse.tile as tile
from concourse.bass_utils import run_bass_kernel_spmd

F32 = mybir.dt.float32
FP16 = mybir.dt.float16
BF16 = mybir.dt.bfloat16

N_CORES = 8
D = 512
BIN = 128
GROUP = 4

LAST_RESULTS = {}


def _plan(lengths, mode="fp16"):
    S = len(lengths)
    n_slots = S // N_CORES
    order = np.argsort(-lengths, kind="stable")
    seg_ids = [[int(order[N_CORES * j + c]) for j in range(n_slots)]
               for c in range(N_CORES)]
    slot_len = [int(lengths[order[N_CORES * j]]) for j in range(n_slots)]

    bins = []   # (used-token count, n_segs) per bin
    slots = []  # (bin, off, L)
    for j, L in enumerate(slot_len):
        bi = next((i for i, (used, ns) in enumerate(bins)
                   if used + L <= BIN and ns < 31), None)
        if bi is None:
            bins.append((0, 0))
            bi = len(bins) - 1
        used, ns = bins[bi]
        slots.append((bi, used, L))
        bins[bi] = (used + L, ns + 1)
    return slots, len(bins), seg_ids


def _groups(n_bins):
    gs = []
    b = 0
    while b < n_bins:
        gs.append((b, min(GROUP, n_bins - b)))
        b += GROUP
    return gs


def _mask_layout(slots, n_bins):
    by_bin = [[] for _ in range(n_bins)]
    for bi, off, L in slots:
        by_bin[bi].append((off, L))
    kmask = [len(by_bin[b]) + 1 for b in range(n_bins)]
    assert max(kmask) <= 32
    return by_bin, kmask


def _build(slots, n_bins, mode="fp16", repeat=1):
    assert mode == "fp16"
    nc = bacc.Bacc("TRN2", target_bir_lowering=False)
    T = n_bins * BIN
    groups = _groups(n_bins)
    n_groups = len(groups)

    by_bin, kmask = _mask_layout(slots, n_bins)

    # cpk is the out-matmul operand (bf16 to match the bf16 exp weights);
    # cpkt is C^T for the u/score matmuls (fp16: score precision matters).
    cpk = nc.dram_tensor("cpk", [T, D], BF16, kind="ExternalInput")
    cpkt = nc.dram_tensor("cpkt", [D, T], FP16, kind="ExternalInput")
    wt = nc.dram_tensor("wt", [128, 4 * D], FP16, kind="ExternalInput")
    bvec = nc.dram_tensor("bvec", [128, 4], F32, kind="ExternalInput")
    # per-group mask rows: bin i of a group at partitions [32i, 32i+km)
    msk = nc.dram_tensor("msk", [n_groups * 128, 2 * 128], FP16,
                         kind="ExternalInput")
    opk = nc.dram_tensor("opk", [T, D], FP16, kind="ExternalOutput")

    ones = nc.inline_tensor(np.ones((128, 1), dtype=ml_dtypes.bfloat16),
                            name="ones")

    with tile.TileContext(nc) as tc:
        with (
            tc.tile_pool(name="const", bufs=1) as cpool,
            tc.tile_pool(name="cb", bufs=3) as cbp,
            tc.tile_pool(name="grp", bufs=3) as grp,
            tc.tile_pool(name="seg", bufs=6) as segp,
            tc.tile_pool(name="stat", bufs=8) as statp,
            tc.tile_pool(name="outp", bufs=3) as outp,
            tc.tile_pool(name="ups", bufs=3, space="PSUM") as ups,
            tc.tile_pool(name="scps", bufs=2, space="PSUM") as scps,
            tc.tile_pool(name="ops", bufs=3, space="PSUM") as opsp,
        ):
            wt_sb = cpool.tile([128, 4, D], FP16, tag="wt")
            b_sb = cpool.tile([128, 4], F32, tag="b")
            ones_sb = cpool.tile([128, 1], BF16, tag="ones")
            msk_sb = cpool.tile([128, n_groups, 2, 128], FP16, tag="msk")
            nc.sync.dma_start(wt_sb[:], wt.ap().rearrange("p (c e) -> p c e", c=4))
            nc.sync.dma_start(b_sb[:], bvec[:])
            nc.sync.dma_start(ones_sb[:], ones[:])
            nc.sync.dma_start(
                msk_sb[:],
                msk.ap().rearrange("(g r) (t p) -> r g t p", g=n_groups, t=2))

            cpk_v = cpk.ap().rearrange("(b p) d -> p b d", p=BIN)
            cpkt_v = cpkt.ap().rearrange("(c p) t -> p c t", p=128)
            opk_v = opk.ap().rearrange("(b p) d -> p b d", p=BIN)

            def load_group(gi):
                b0, gs = groups[gi]
                cg = cbp.tile([128, gs, D], BF16, tag="cg")
                nc.sync.dma_start(cg[:], cpk_v[:, b0:b0 + gs, :])
                ct = grp.tile([128, 4, gs * BIN], FP16, tag="ct")
                nc.sync.dma_start(
                    ct[:], cpkt_v[:, :, b0 * BIN:(b0 + gs) * BIN])
                return {"gi": gi, "cg": cg, "ct": ct}

            def u_group(st):
                _, gs = groups[st["gi"]]
                ct = st["ct"]
                ut = grp.tile([128, 4, gs * BIN], FP16, tag="ut")
                for c in range(4):
                    ups_t = ups.tile([128, gs * BIN], F32, tag="ups")
                    for k in range(4):
                        nc.tensor.matmul(
                            ups_t[:], wt_sb[:, k, c * 128:(c + 1) * 128],
                            ct[:, k, :], start=(k == 0), stop=(k == 3))
                    nc.scalar.activation(
                        ut[:, c, :], ups_t[:],
                        mybir.ActivationFunctionType.Tanh, bias=b_sb[:, c:c + 1])
                st["ut"] = ut

            def bin_scores(st, i):
                gi = st["gi"]
                b0, gs = groups[gi]
                b = b0 + i
                if not by_bin[b]:
                    return
                ct, ut = st["ct"], st["ut"]
                km = kmask[b]
                # transposed scores: sc_T[k, q] = sum_e u^T[e,k] C^T[e,q]
                sc = scps.tile([128, 128], F32, tag="sc")
                for c in range(4):
                    nc.tensor.matmul(
                        sc[:], ut[:, c, i * BIN:(i + 1) * BIN],
                        ct[:, c, i * BIN:(i + 1) * BIN],
                        start=(c == 0), stop=False)
                nc.tensor.matmul(sc[:], msk_sb[32 * i:32 * i + km, gi, 1, :],
                                 msk_sb[32 * i:32 * i + km, gi, 0, :],
                                 start=False, stop=True,
                                 tile_position=(32 * i, 0))
                # max-free softmax numerator, already [k, q] for the out-matmul
                attn = segp.tile([128, 128], BF16, tag="attn")
                nc.scalar.activation(
                    attn[:], sc[:], mybir.ActivationFunctionType.Exp)
                st[("attn", i)] = attn

            def bin_out(st, i, og, use_act_copy):
                if ("attn", i) not in st:
                    return
                attn = st.pop(("attn", i))
                cg = st["cg"]
                sums = scps.tile([128, 1], F32, tag="sc")
                nc.tensor.matmul(sums[:], attn[:], ones_sb[:],
                                 start=True, stop=True)
                ops_t = opsp.tile([128, D], F32, tag="ops")
                nc.tensor.matmul(ops_t[:], attn[:], cg[:, i, :],
                                 start=True, stop=True)
                recip = statp.tile([128, 1], F32, tag="recip")
                nc.vector.reciprocal(recip[:], sums[:])
                # normalize rows by 1/sum during the psum->sbuf copy
                if use_act_copy:
                    nc.scalar.activation(og[:, i, :], ops_t[:],
                                         mybir.ActivationFunctionType.Copy,
                                         scale=recip[:])
                else:
                    nc.vector.tensor_scalar_mul(og[:, i, :], ops_t[:], recip[:])

            def finish_group(st):
                gi = st["gi"]
                b0, gs = groups[gi]
                og = outp.tile([128, gs, D], FP16, tag="og")
                for i in range(gs):
                    bin_out(st, i, og, use_act_copy=(i % 2 == 0))
                # ACT HWDGE queue: keeps the blocking store off the SP
                # load queue (HWDGE DMAs issue in order per engine queue)
                nc.scalar.dma_start(opk_v[:, b0:b0 + gs, :], og[:])

            # software pipeline over groups, prefetch depth 2:
            #   iter t: load group t | u-matmuls group t-1 | scores/out t-2
            niter = repeat * n_groups
            states = {}
            for it in range(niter + 2):
                st_s = states.pop(it - 2, None)
                if st_s is not None:
                    for i in range(groups[st_s["gi"]][1]):
                        bin_scores(st_s, i)
                st_u = states.get(it - 1)
                if st_u is not None:
                    u_group(st_u)
                if st_s is not None:
                    finish_group(st_s)
                if it < niter:
                    states[it] = load_group(it % n_groups)

    nc.compile()
    return nc


def _host_arrays(slots, n_bins, seg_ids, lengths, context, W, b, mode="fp16"):
    NEG = -30000.0
    # uniform score shift (softmax-invariant, fp16-exact since 30016 is
    # representable): centers the max-free exp range, keeping e^s comfortably
    # inside fp32/bf16 for score outliers in either direction.
    SHIFT = 16.0
    T = n_bins * BIN
    by_bin2 = [[] for _ in range(n_bins)]
    for j, (bi, off, L) in enumerate(slots):
        by_bin2[bi].append((j, off, L))
    n_groups = len(_groups(n_bins))

    wt = np.ascontiguousarray(
        W.T.reshape(4, 128, D).transpose(1, 0, 2).reshape(128, 4 * D)
    ).astype(np.float16)
    bvec = np.ascontiguousarray(b.reshape(4, 128).T).astype(np.float32)

    in_maps = []
    for c in range(N_CORES):
        cpk = np.zeros((T, D), np.float16)
        kb = np.full(T, NEG, np.float32)
        for j, (bi, off, _L) in enumerate(slots):
            s = seg_ids[c][j]
            n = int(lengths[s])
            r0 = bi * BIN + off
            cpk[r0:r0 + n] = context[s, :n].astype(np.float16)
            kb[r0:r0 + n] = 0.0
        msk = np.zeros((n_groups * 128, 2, 128), np.float32)
        for bb in range(n_bins):
            r0 = (bb // GROUP) * 128 + 32 * (bb % GROUP)
            msk[r0, 0] = 1.0
            msk[r0, 1] = kb[bb * BIN:(bb + 1) * BIN] + NEG - SHIFT
            for r, (_j, off, L) in enumerate(by_bin2[bb]):
                msk[r0 + 1 + r, 0, off:off + L] = 1.0
                msk[r0 + 1 + r, 1, off:off + L] = -NEG
        in_maps.append({"cpk": cpk.astype(ml_dtypes.bfloat16),
                        "cpkt": np.ascontiguousarray(cpk.T),
                        "wt": wt, "bvec": bvec,
                        "msk": msk.reshape(n_groups * 128, 256).astype(np.float16)})
    return in_maps


_CACHE = {}


def kernel(context, lengths, W, b, mode="fp16"):
    context = np.asarray(context, dtype=np.float32)
    lengths = np.asarray(lengths, dtype=np.int32)
    W = np.asarray(W, dtype=np.float32)
    b = np.asarray(b, dtype=np.float32)
    S, Lmax, Din = context.shape

    slots, n_bins, seg_ids = _plan(lengths, mode)
    key = (tuple(slots), n_bins, mode)
    if key in _CACHE:
        nc = _CACHE[key]
    else:
        nc = _build(slots, n_bins, mode)
        _CACHE[key] = nc

    in_maps = _host_arrays(slots, n_bins, seg_ids, lengths, context, W, b, mode)
    res = run_bass_kernel_spmd(nc, in_maps, list(range(N_CORES)))
    LAST_RESULTS["exec_time_ns"] = res.exec_time_ns

    out = np.zeros((S, Lmax, D), np.float32)
    for c in range(N_CORES):
        opk = res.results[c]["opk"].astype(np.float32)
        for j, (bi, off, _L) in enumerate(slots):
            s = seg_ids[c][j]
            n = int(lengths[s])
            r0 = bi * BIN + off
            out[s, :n] = opk[r0:r0 + n]
    return out
